# revision 19
# baseline (speedup 1.0000x reference)
"""Trainium2 Bass kernel for nn_CLIPVisionTowerContextMoe.

Data-parallel over batch B=16 across 8 NeuronCores (2 batches/core, no
collectives). All matmuls in float32r (TF32-like, 1 cyc/row at N>=256).
Feature-major layouts throughout; host pre-transposes weights/activations.
Tokens padded 577 -> 640 per batch (clean 512/256 moving tiles).

Per-core pipeline:
  gate path:  txt resblock -> q/k-bias/v-bias -> per-320-token-tile img
              resblock -> k+scores / v+online-attention (unnormalized exp,
              no max pass) -> task -> gate logits/probs
  expert path: 32 phases of (expert e, 4-f-chunk block): MM1 + fused
              bias+QuickGELU (Sigmoid ACT + scalar_tensor_tensor) ->
              token-major MM2 (swapped operands) -> fused scaled flush
              out_acc += g[b,e] * psum.  Expert bias b2 folded in via
              out_acc init from broadcast gp@exp_b2 rows.
"""
import os
import sys

# The kernel executes through the axon PJRT backend; a JAX_PLATFORMS=cpu pin
# (common for reference computation) would hide the 8 NeuronCores. Clear it
# before jax's backend initializes (no-op if jax is already initialized).
if os.environ.get("JAX_PLATFORMS") == "cpu" and "jax" not in sys.modules:
    os.environ.pop("JAX_PLATFORMS")

import numpy as np
from contextlib import ExitStack

import concourse.bacc as bacc
import concourse.mybir as mybir
import concourse.tile as tile
from concourse.bass_utils import run_bass_kernel_spmd

F32 = mybir.dt.float32
F32R = mybir.dt.float32r
AF = mybir.ActivationFunctionType
OP = mybir.AluOpType
AX = mybir.AxisListType

B, N, D, C, E, F, H = 16, 577, 1024, 768, 4, 4096, 4
NPAD, BL = 640, 2
T = BL * NPAD            # 1280
HD = C // H              # 192
F2, C2 = 2 * D, 2 * C    # 2048, 1536
NC_ = 8                  # cores
DC_, CC_, FC_, F2C, C2C = D // 128, C // 128, F // 128, F2 // 128, C2 // 128
NT = 320                 # gate-phase token tile (4 tiles, no batch crossing)
SCALE_ATT = 1.0 / np.sqrt(HD)
QG = 1.702
TBLK = [(0, 512), (512, 1024), (1024, 1280)]   # expert MM1 moving tiles
NTC = T // 128           # 10 token chunks for MM2/out
NF = 4                   # f-chunks per expert phase


def _build():
    nc = bacc.Bacc("TRN2", num_devices=NC_)
    dt = nc.dram_tensor

    # ---- inputs (per core). f32r for matmul-consumed tensors. ----
    xt = dt("xt", [D, T], F32R, kind="ExternalInput")
    ctxt = dt("ctxt", [C, BL], F32R, kind="ExternalInput")
    w1i = dt("w1i", [D, F2], F32R, kind="ExternalInput")      # img_fc1_w.T
    w2i = dt("w2i", [F2, D], F32R, kind="ExternalInput")      # img_fc2_w.T
    w1t = dt("w1t", [C, C2], F32R, kind="ExternalInput")
    w2t = dt("w2t", [C2, C], F32R, kind="ExternalInput")
    qw = dt("qw", [C, C], F32R, kind="ExternalInput")
    kwi = dt("kwi", [D, C], F32R, kind="ExternalInput")
    kwt = dt("kwt", [C, C], F32R, kind="ExternalInput")
    vwi = dt("vwi", [D, C], F32R, kind="ExternalInput")
    vwt = dt("vwt", [C, C], F32R, kind="ExternalInput")
    aow = dt("aow", [C, C], F32R, kind="ExternalInput")
    gw = dt("gw", [C, E], F32R, kind="ExternalInput")
    ew1 = dt("ew1", [E, D, F], F32R, kind="ExternalInput")    # exp_w1 transposed
    ew2 = dt("ew2", [E, F, D], F32R, kind="ExternalInput")    # exp_w2 transposed
    b2stack = dt("b2stack", [E, D], F32R, kind="ExternalInput")
    b1i_c = dt("b1i_c", [128, F2C], F32, kind="ExternalInput")
    b1i_s = dt("b1i_s", [128, F2C], F32, kind="ExternalInput")
    b2i_c = dt("b2i_c", [128, DC_], F32, kind="ExternalInput")
    gi_c = dt("gi_c", [128, DC_], F32, kind="ExternalInput")
    bi_c = dt("bi_c", [128, DC_], F32, kind="ExternalInput")
    b1t_c = dt("b1t_c", [128, C2C], F32, kind="ExternalInput")
    b1t_s = dt("b1t_s", [128, C2C], F32, kind="ExternalInput")
    b2t_c = dt("b2t_c", [128, CC_], F32, kind="ExternalInput")
    gt_c = dt("gt_c", [128, CC_], F32, kind="ExternalInput")
    bt_c = dt("bt_c", [128, CC_], F32, kind="ExternalInput")
    qb_c = dt("qb_c", [128, CC_], F32, kind="ExternalInput")
    kb_c = dt("kb_c", [128, CC_], F32, kind="ExternalInput")
    vb_c = dt("vb_c", [128, CC_], F32, kind="ExternalInput")
    aob_c = dt("aob_c", [128, CC_], F32, kind="ExternalInput")
    gb_r = dt("gb_r", [BL, E], F32, kind="ExternalInput")
    eb1_c = dt("eb1_c", [128, E, FC_], F32, kind="ExternalInput")
    eb1_s = dt("eb1_s", [128, E, FC_], F32, kind="ExternalInput")
    ones_col_d = dt("ones_col", [128, 1], F32R, kind="ExternalInput")
    ones_row_d = dt("ones_row", [1, 128], F32R, kind="ExternalInput")
    ind4_d = dt("ind4", [128, CC_, H], F32, kind="ExternalInput")
    ind4t_d = dt("ind4t", [H, C], F32R, kind="ExternalInput")

    out = dt("out", [BL, N, D], F32, kind="ExternalOutput")
    gate_out = dt("gate_out", [BL, E], F32, kind="ExternalOutput")

    scr_gp = dt("scr_gp", [BL, E], F32R, kind="Internal")
    scr_b2m = dt("scr_b2m", [BL, D], F32R, kind="Internal")

    with nc.allow_low_precision("float32r tiles feed f32r matmuls by design"), \
            tile.TileContext(nc) as tc, \
            ExitStack() as ctx:
        persist = ctx.enter_context(tc.tile_pool(name="persist", bufs=1))
        big = ctx.enter_context(tc.tile_pool(name="big", bufs=1))
        w2rp = ctx.enter_context(tc.tile_pool(name="w2rp", bufs=1))
        stream = ctx.enter_context(tc.tile_pool(name="stream", bufs=4))
        gbuf = ctx.enter_context(tc.tile_pool(name="gbuf", bufs=1))
        trans = ctx.enter_context(tc.tile_pool(name="trans", bufs=2))
        ps = ctx.enter_context(tc.tile_pool(name="ps", bufs=8, space="PSUM"))

        def pst(p_, n_):
            return ps.tile([p_, n_], F32, tag="ps", name="pst")

        # ---- resident loads ----
        xt_sb = persist.tile([128, DC_, T], F32R)
        nc.sync.dma_start(xt_sb, xt.ap().rearrange("(dc p) t -> p dc t", p=128))
        out_acc = persist.tile([128, NTC, D], F32)
        eb1c_sb = persist.tile([128, E, FC_], F32)
        nc.sync.dma_start(eb1c_sb, eb1_c.ap())
        eb1s_sb = persist.tile([128, E, FC_], F32)
        nc.sync.dma_start(eb1s_sb, eb1_s.ap())
        ones_col = persist.tile([128, 1], F32R)
        nc.sync.dma_start(ones_col, ones_col_d.ap())
        ones_row = persist.tile([1, 128], F32R)
        nc.sync.dma_start(ones_row, ones_row_d.ap())
        ind4 = persist.tile([128, CC_, H], F32)
        nc.sync.dma_start(ind4, ind4_d.ap())
        ind4t = persist.tile([H, C], F32R)
        nc.sync.dma_start(ind4t, ind4t_d.ap())
        gw_sb = persist.tile([128, CC_, E], F32R)
        nc.sync.dma_start(gw_sb, gw.ap().rearrange("(cc p) e -> p cc e", p=128))
        gb_sb = persist.tile([BL, E], F32)
        nc.sync.dma_start(gb_sb, gb_r.ap())
        b2stack_sb = persist.tile([E, D], F32R)
        nc.sync.dma_start(b2stack_sb, b2stack.ap())

        def load_cm(name, dram, nch):
            t_ = persist.tile([128, nch], F32, tag=name, name=name)
            nc.sync.dma_start(t_, dram.ap())
            return t_

        b1ic = load_cm("b1ic", b1i_c, F2C); b1is = load_cm("b1is", b1i_s, F2C)
        b2ic = load_cm("b2ic", b2i_c, DC_)
        gic = load_cm("gic", gi_c, DC_); bic = load_cm("bic", bi_c, DC_)
        b1tc = load_cm("b1tc", b1t_c, C2C); b1ts = load_cm("b1ts", b1t_s, C2C)
        b2tc = load_cm("b2tc", b2t_c, CC_)
        gtc = load_cm("gtc", gt_c, CC_); btc = load_cm("btc", bt_c, CC_)
        qbc = load_cm("qbc", qb_c, CC_); kbc = load_cm("kbc", kb_c, CC_)
        vbc = load_cm("vbc", vb_c, CC_); aobc = load_cm("aobc", aob_c, CC_)

        kbias = persist.tile([128, CC_, BL], F32)
        vbias = persist.tile([128, CC_, BL], F32)
        qh = persist.tile([128, CC_, BL, H], F32R)
        sumexp = persist.tile([H, BL], F32)
        av = persist.tile([128, CC_, BL], F32)
        gbc = persist.tile([128, BL * E], F32)
        nc.vector.memset(sumexp, 0.0)
        nc.vector.memset(av, 0.0)
        eps_t = persist.tile([128, 1], F32)
        nc.vector.memset(eps_t, 1e-5)

        def stream_w(dram_ap, kch, col0, ncols=128, k0=0):
            """lhsT block [128, kch, ncols] from a [K, M] dram weight (shared tag)."""
            t_ = stream.tile([128, kch, ncols], F32R, tag="ws", name="ws")
            nc.sync.dma_start(
                t_, dram_ap.rearrange("(kc p) m -> p kc m", p=128)
                [:, k0:k0 + kch, col0:col0 + ncols])
            return t_

        # ============ txt resblock + q/kbias/vbias (feature-major, N=2) ============
        ctx_sb = persist.tile([128, CC_, BL], F32R)
        nc.sync.dma_start(ctx_sb, ctxt.ap().rearrange("(cc p) b -> p cc b", p=128))

        ht = trans.tile([128, C2C, BL], F32R, tag="ht")
        for c2 in range(C2C):
            w_ = stream_w(w1t.ap(), CC_, c2 * 128)
            z = pst(128, BL)
            for cc in range(CC_):
                nc.tensor.matmul(z, w_[:, cc, :], ctx_sb[:, cc, :],
                                 start=(cc == 0), stop=(cc == CC_ - 1))
            s_ = trans.tile([128, BL], F32, tag="sig")
            nc.scalar.activation(s_, z, AF.Sigmoid, bias=b1ts[:, c2:c2 + 1], scale=QG)
            nc.vector.scalar_tensor_tensor(ht[:, c2, :], z, b1tc[:, c2:c2 + 1], s_,
                                           op0=OP.add, op1=OP.mult)
        yt = trans.tile([128, CC_, BL], F32R, tag="yt")
        ps_s = pst(1, BL)
        ps_q = pst(1, BL)
        for cc in range(CC_):
            z = pst(128, BL)
            for ch in range(2):
                w_ = stream_w(w2t.ap(), C2C // 2, cc * 128, k0=ch * (C2C // 2))
                for c2 in range(C2C // 2):
                    nc.tensor.matmul(z, w_[:, c2, :], ht[:, ch * (C2C // 2) + c2, :],
                                     start=(ch == 0 and c2 == 0),
                                     stop=(ch == 1 and c2 == C2C // 2 - 1))
            nc.vector.scalar_tensor_tensor(yt[:, cc, :], z, b2tc[:, cc:cc + 1],
                                           ctx_sb[:, cc, :], op0=OP.add, op1=OP.add)
            sqt = trans.tile([128, BL], F32R, tag="sqt")
            nc.scalar.activation(sqt, yt[:, cc, :], AF.Square)
            nc.tensor.matmul(ps_s, ones_col, yt[:, cc, :],
                             start=(cc == 0), stop=(cc == CC_ - 1))
            nc.tensor.matmul(ps_q, ones_col, sqt,
                             start=(cc == 0), stop=(cc == CC_ - 1))

        def ln_rows(ps_s_, ps_q_, nfeat, width):
            """(rstd_bc, nb_bc) psum [128, width] from sum/sumsq row psums."""
            m_ = trans.tile([1, width], F32, tag="lnr1", name="m_")
            nc.scalar.mul(m_, ps_s_, 1.0 / nfeat)
            m2 = trans.tile([1, width], F32, tag="lnr2", name="m2")
            nc.vector.tensor_mul(m2, m_, m_)
            var = trans.tile([1, width], F32, tag="lnr3", name="var")
            nc.vector.scalar_tensor_tensor(var, ps_q_, 1.0 / nfeat, m2,
                                           op0=OP.mult, op1=OP.subtract)
            sd = trans.tile([1, width], F32, tag="lnr4", name="sd")
            nc.scalar.activation(sd, var, AF.Sqrt, bias=eps_t[0:1, :])
            rstd = trans.tile([1, width], F32R, tag="lnr5", name="rstd")
            nc.vector.reciprocal(rstd, sd)
            nb = trans.tile([1, width], F32R, tag="lnr6", name="nb")
            nc.vector.scalar_tensor_tensor(nb, m_, -1.0, rstd, op0=OP.mult, op1=OP.mult)
            bc_r = pst(128, width)
            nc.tensor.matmul(bc_r, ones_row, rstd, start=True, stop=True)
            bc_n = pst(128, width)
            nc.tensor.matmul(bc_n, ones_row, nb, start=True, stop=True)
            return bc_r, bc_n

        bc_r, bc_n = ln_rows(ps_s, ps_q, C, BL)
        txt_sb = persist.tile([128, CC_, BL], F32R)
        for cc in range(CC_):
            t1 = trans.tile([128, BL], F32, tag="t1")
            nc.vector.tensor_mul(t1, yt[:, cc, :], bc_r)
            nc.vector.tensor_add(t1, t1, bc_n)
            nc.vector.tensor_scalar(txt_sb[:, cc, :], t1, gtc[:, cc:cc + 1],
                                    btc[:, cc:cc + 1], op0=OP.mult, op1=OP.add)

        def proj_c(w_dram, bias_cm, out_t):
            for co in range(CC_):
                w_ = stream_w(w_dram.ap(), CC_, co * 128)
                z = pst(128, BL)
                for ci in range(CC_):
                    nc.tensor.matmul(z, w_[:, ci, :], txt_sb[:, ci, :],
                                     start=(ci == 0), stop=(ci == CC_ - 1))
                nc.scalar.activation(out_t[:, co, :], z, AF.Identity,
                                     bias=bias_cm[:, co:co + 1])

        qt = persist.tile([128, CC_, BL], F32)
        proj_c(qw, qbc, qt)
        proj_c(kwt, kbc, kbias)
        proj_c(vwt, vbc, vbias)

        for cc in range(CC_):
            for b in range(BL):
                nc.vector.tensor_scalar_mul(qh[:, cc, b, :], ind4[:, cc, :],
                                            qt[:, cc, b:b + 1])

        # ============ img resblock + k/scores + v/attention per 320-token tile ========
        for it in range(4):
            b = it // 2
            t0 = it * NT
            xsl = xt_sb[:, :, t0:t0 + NT]
            real = NT if it % 2 == 0 else N - NT   # 320 or 257

            h_img = big.tile([128, F2C, NT], F32R, tag="big", name="h_img")
            for f2 in range(F2C):
                w_ = stream_w(w1i.ap(), DC_, f2 * 128)
                z = pst(128, NT)
                for dc in range(DC_):
                    nc.tensor.matmul(z, w_[:, dc, :], xsl[:, dc, :],
                                     start=(dc == 0), stop=(dc == DC_ - 1))
                s_ = trans.tile([128, NT], F32, tag="sigi")
                nc.scalar.activation(s_, z, AF.Sigmoid, bias=b1is[:, f2:f2 + 1], scale=QG)
                zb = trans.tile([128, 512], F32, tag="zb", name="zbi")
                nc.scalar.activation(zb[:, :NT], z, AF.Identity, bias=b1ic[:, f2:f2 + 1])
                nc.vector.tensor_mul(h_img[:, f2, :], zb[:, :NT], s_)

            y = gbuf.tile([128, DC_, NT], F32R, tag="y_img", name="y")
            ps_s2 = pst(1, NT)
            ps_q2 = pst(1, NT)
            for dc in range(DC_):
                z = pst(128, NT)
                for ch in range(2):
                    wa = stream_w(w2i.ap(), F2C // 2, dc * 128, k0=ch * (F2C // 2))
                    for f2 in range(F2C // 2):
                        nc.tensor.matmul(z, wa[:, f2, :],
                                         h_img[:, ch * (F2C // 2) + f2, :],
                                         start=(ch == 0 and f2 == 0),
                                         stop=(ch == 1 and f2 == F2C // 2 - 1))
                nc.vector.scalar_tensor_tensor(y[:, dc, :], z, b2ic[:, dc:dc + 1],
                                               xsl[:, dc, :], op0=OP.add, op1=OP.add)
                sq = trans.tile([128, NT], F32R, tag="sq")
                nc.scalar.activation(sq, y[:, dc, :], AF.Square)
                nc.tensor.matmul(ps_s2, ones_col, y[:, dc, :],
                                 start=(dc == 0), stop=(dc == DC_ - 1))
                nc.tensor.matmul(ps_q2, ones_col, sq,
                                 start=(dc == 0), stop=(dc == DC_ - 1))
            bc_r2, bc_n2 = ln_rows(ps_s2, ps_q2, D, NT)
            imgn = gbuf.tile([128, DC_, NT], F32R, tag="imgn", name="imgn")
            for dc in range(DC_):
                t2 = trans.tile([128, NT], F32, tag="t2")
                nc.vector.tensor_mul(t2, y[:, dc, :], bc_r2)
                nc.vector.tensor_add(t2, t2, bc_n2)
                nc.vector.tensor_scalar(imgn[:, dc, :], t2, gic[:, dc:dc + 1],
                                        bic[:, dc:dc + 1], op0=OP.mult, op1=OP.add)

            # pass 1: k per c-chunk + scores accumulation
            z_sc = pst(H, NT)
            for cc in range(CC_):
                w_ = stream_w(kwi.ap(), DC_, cc * 128)
                zk = pst(128, NT)
                for dc in range(DC_):
                    nc.tensor.matmul(zk, w_[:, dc, :], imgn[:, dc, :],
                                     start=(dc == 0), stop=(dc == DC_ - 1))
                k_cc = trans.tile([128, NT], F32R, tag="k_cc")
                nc.scalar.activation(k_cc, zk, AF.Identity, bias=kbias[:, cc, b:b + 1])
                nc.tensor.matmul(z_sc, qh[:, cc, b, :], k_cc,
                                 start=(cc == 0), stop=(cc == CC_ - 1))
            ex = trans.tile([H, NT], F32R, tag="ex")
            nc.scalar.activation(ex, z_sc, AF.Exp, scale=SCALE_ATT)
            red4 = trans.tile([H, 1], F32, tag="red4")
            nc.vector.tensor_reduce(red4, ex[:, :real], axis=AX.X, op=OP.add)
            nc.vector.tensor_add(sumexp[:, b:b + 1], sumexp[:, b:b + 1], red4)

            # pass 2: v per c-chunk + online attention-value accumulation
            for cc in range(CC_):
                w_ = stream_w(vwi.ap(), DC_, cc * 128)
                zv = pst(128, NT)
                for dc in range(DC_):
                    nc.tensor.matmul(zv, w_[:, dc, :], imgn[:, dc, :],
                                     start=(dc == 0), stop=(dc == DC_ - 1))
                v_cc = trans.tile([128, NT], F32R, tag="v_cc")
                nc.scalar.activation(v_cc, zv, AF.Identity, bias=vbias[:, cc, b:b + 1])
                zb = pst(128, NT)
                nc.tensor.matmul(zb, ind4t[:, cc * 128:(cc + 1) * 128],
                                 ex, start=True, stop=True)
                prod = trans.tile([128, NT], F32, tag="prod")
                nc.vector.tensor_mul(prod[:, :real], v_cc[:, :real], zb[:, :real])
                red = trans.tile([128, 1], F32, tag="red")
                nc.vector.tensor_reduce(red, prod[:, :real], axis=AX.X, op=OP.add)
                nc.vector.tensor_add(av[:, cc, b:b + 1], av[:, cc, b:b + 1], red)

        # ============ attention tail: avn -> task -> gate ============
        recip = trans.tile([H, BL], F32R, tag="recip")
        nc.vector.reciprocal(recip, sumexp)
        avn = persist.tile([128, CC_, BL], F32R)
        for cc in range(CC_):
            zr = pst(128, BL)
            nc.tensor.matmul(zr, ind4t[:, cc * 128:(cc + 1) * 128], recip,
                             start=True, stop=True)
            nc.vector.tensor_mul(avn[:, cc, :], av[:, cc, :], zr)
        taskT = persist.tile([128, CC_, BL], F32R)
        for co in range(CC_):
            w_ = stream_w(aow.ap(), CC_, co * 128)
            z = pst(128, BL)
            for ci in range(CC_):
                nc.tensor.matmul(z, w_[:, ci, :], avn[:, ci, :],
                                 start=(ci == 0), stop=(ci == CC_ - 1))
            nc.scalar.activation(taskT[:, co, :], z, AF.Identity,
                                 bias=aobc[:, co:co + 1])
        z_g = pst(BL, E)
        for cc in range(CC_):
            nc.tensor.matmul(z_g, taskT[:, cc, :], gw_sb[:, cc, :],
                             start=(cc == 0), stop=(cc == CC_ - 1))
        gl = trans.tile([BL, E], F32, tag="gl")
        nc.vector.tensor_add(gl, z_g, gb_sb)
        nc.sync.dma_start(gate_out.ap(), gl)
        uexp = trans.tile([BL, E], F32, tag="uexp")
        nc.scalar.activation(uexp, gl, AF.Exp)
        srow = trans.tile([BL, 1], F32, tag="srow")
        nc.vector.tensor_reduce(srow, uexp, axis=AX.X, op=OP.add)
        rrow = trans.tile([BL, 1], F32, tag="rrow")
        nc.vector.reciprocal(rrow, srow)
        gp = trans.tile([BL, E], F32R, tag="gp")
        nc.vector.tensor_scalar_mul(gp, uexp, rrow)
        nc.sync.dma_start(scr_gp.ap(), gp)
        gp_row = trans.tile([1, BL, E], F32R, tag="gp_row")
        nc.sync.dma_start(gp_row, scr_gp.ap().rearrange("b e -> () b e"))
        gpT = trans.tile([E, BL], F32R, tag="gpT")
        nc.sync.dma_start(gpT, scr_gp.ap().rearrange("b e -> e b"))
        z_gbc = pst(128, BL * E)
        nc.tensor.matmul(z_gbc, ones_row, gp_row, start=True, stop=True)
        nc.scalar.copy(gbc, z_gbc)
        b2m = gbuf.tile([BL, D], F32R, tag="y_img", name="b2m")
        for dh in range(2):
            z_b2 = pst(BL, 512)
            nc.tensor.matmul(z_b2, gpT, b2stack_sb[:, dh * 512:(dh + 1) * 512],
                             start=True, stop=True)
            nc.scalar.copy(b2m[:, dh * 512:(dh + 1) * 512], z_b2)
        nc.sync.dma_start(scr_b2m.ap(), b2m)
        b2row = gbuf.tile([1, BL, D], F32R, tag="imgn", name="b2row")
        nc.sync.dma_start(b2row, scr_b2m.ap().rearrange("b d -> () b d"))
        for q4 in range(4):
            b, dh = q4 // 2, q4 % 2
            z_bb = pst(128, 512)
            nc.tensor.matmul(z_bb, ones_row, b2row[:, b, dh * 512:dh * 512 + 512],
                             start=True, stop=True)
            for tc_ in range(b * (NTC // 2), (b + 1) * (NTC // 2)):
                nc.vector.tensor_copy(out_acc[:, tc_, dh * 512:(dh + 1) * 512], z_bb)

        # ============ experts: 32 phases of (e, 4-f-chunk block) ============
        for e in range(E):
            for fh in range(FC_ // NF):
                h_exp = big.tile([128, NF, T], F32R, tag="big", name="h_exp")
                w2r = w2rp.tile([128, NF, D], F32R, name="w2r")
                nc.sync.dma_start(
                    w2r, ew2.ap()[e].rearrange("(fb p) d -> p fb d", p=128)
                    [:, fh * NF:(fh + 1) * NF, :])
                for fi in range(NF):
                    fc = fh * NF + fi
                    w_ = stream_w(ew1.ap()[e], DC_, fc * 128)
                    zs = [pst(128, tb - ta) for (ta, tb) in TBLK]
                    for dc in range(DC_):
                        for k, (ta, tb) in enumerate(TBLK):
                            nc.tensor.matmul(zs[k], w_[:, dc, :], xt_sb[:, dc, ta:tb],
                                             start=(dc == 0), stop=(dc == DC_ - 1))
                    for k, (ta, tb) in enumerate(TBLK):
                        s_ = trans.tile([128, 512], F32, tag="sige")
                        nc.scalar.activation(s_[:, :tb - ta], zs[k], AF.Sigmoid,
                                             bias=eb1s_sb[:, e, fc:fc + 1], scale=QG)
                        zb = trans.tile([128, 512], F32, tag="zb", name="zbe")
                        nc.scalar.activation(zb[:, :tb - ta], zs[k], AF.Identity,
                                             bias=eb1c_sb[:, e, fc:fc + 1])
                        nc.vector.tensor_mul(h_exp[:, fi, ta:tb], zb[:, :tb - ta],
                                             s_[:, :tb - ta])
                for tc_ in range(NTC):
                    b = tc_ // (NTC // 2)
                    gsc = gbc[:, b * E + e:b * E + e + 1]
                    for dh in range(2):
                        z = pst(128, 512)
                        for fi in range(NF):
                            nc.tensor.matmul(z, h_exp[:, fi, tc_ * 128:(tc_ + 1) * 128],
                                             w2r[:, fi, dh * 512:(dh + 1) * 512],
                                             start=(fi == 0), stop=(fi == NF - 1))
                        sl = out_acc[:, tc_, dh * 512:(dh + 1) * 512]
                        nc.vector.scalar_tensor_tensor(sl, z, gsc, sl,
                                                       op0=OP.mult, op1=OP.add)

        # ---- write outputs ----
        for tc_ in range(NTC):
            b = tc_ // (NTC // 2)
            n0 = (tc_ % (NTC // 2)) * 128
            rows = min(128, N - n0)
            if rows <= 0:
                continue
            nc.sync.dma_start(out.ap()[b, n0:n0 + rows, :], out_acc[:rows, tc_, :])

    nc.finalize()
    return nc


_NC_CACHE = {}


def _get_nc():
    if "nc" not in _NC_CACHE:
        _NC_CACHE["nc"] = _build()
    return _NC_CACHE["nc"]


def _prep_shared(w):
    """Host-side weight marshalling (shared across cores)."""
    f32 = np.float32
    c = lambda a: np.ascontiguousarray(a, dtype=f32)
    cm = lambda v, nch: c(np.asarray(v, f32).reshape(nch, 128).T)
    head_of = np.arange(C) // HD
    ind4 = np.zeros((128, CC_, H), f32)
    for cc in range(CC_):
        p = np.arange(128) + cc * 128
        ind4[np.arange(128), cc, head_of[p]] = 1.0
    ind4t = np.zeros((H, C), f32)
    ind4t[head_of, np.arange(C)] = 1.0
    return {
        "w1i": c(np.asarray(w["img_fc1_w"]).T), "w2i": c(np.asarray(w["img_fc2_w"]).T),
        "w1t": c(np.asarray(w["txt_fc1_w"]).T), "w2t": c(np.asarray(w["txt_fc2_w"]).T),
        "qw": c(np.asarray(w["q_w"]).T),
        "kwi": c(np.asarray(w["k_w"])[:, :D].T), "kwt": c(np.asarray(w["k_w"])[:, D:].T),
        "vwi": c(np.asarray(w["v_w"])[:, :D].T), "vwt": c(np.asarray(w["v_w"])[:, D:].T),
        "aow": c(np.asarray(w["attn_out_w"]).T), "gw": c(np.asarray(w["gate_w"]).T),
        "ew1": c(np.asarray(w["exp_w1"]).transpose(0, 2, 1)),
        "ew2": c(np.asarray(w["exp_w2"]).transpose(0, 2, 1)),
        "b2stack": c(w["exp_b2"]),
        "b1i_c": cm(w["img_fc1_b"], F2C), "b1i_s": cm(np.asarray(w["img_fc1_b"]) * QG, F2C),
        "b2i_c": cm(w["img_fc2_b"], DC_),
        "gi_c": cm(w["img_ln_g"], DC_), "bi_c": cm(w["img_ln_b"], DC_),
        "b1t_c": cm(w["txt_fc1_b"], C2C), "b1t_s": cm(np.asarray(w["txt_fc1_b"]) * QG, C2C),
        "b2t_c": cm(w["txt_fc2_b"], CC_),
        "gt_c": cm(w["txt_ln_g"], CC_), "bt_c": cm(w["txt_ln_b"], CC_),
        "qb_c": cm(w["q_b"], CC_), "kb_c": cm(w["k_b"], CC_), "vb_c": cm(w["v_b"], CC_),
        "aob_c": cm(w["attn_out_b"], CC_),
        "gb_r": c(np.broadcast_to(np.asarray(w["gate_b"], f32)[None, :], (BL, E))),
        "eb1_c": c(np.asarray(w["exp_b1"], f32).reshape(E, FC_, 128).transpose(2, 0, 1)),
        "eb1_s": c((np.asarray(w["exp_b1"], f32) * QG).reshape(E, FC_, 128).transpose(2, 0, 1)),
        "ones_col": np.ones((128, 1), f32), "ones_row": np.ones((1, 128), f32),
        "ind4": ind4, "ind4t": ind4t,
    }


_LAST_EXEC_NS = None


def kernel(**inputs):
    global _LAST_EXEC_NS
    nc = _get_nc()
    shared = _prep_shared(inputs)
    hs = np.asarray(inputs["hidden_states"], np.float32)
    ce = np.asarray(inputs["context_embeddings"], np.float32)
    in_maps = []
    for core in range(NC_):
        b0 = core * BL
        xT = np.zeros((D, T), np.float32)
        for b in range(BL):
            xT[:, b * NPAD:b * NPAD + N] = hs[b0 + b].T
        m = dict(shared)
        m["xt"] = xT
        m["ctxt"] = np.ascontiguousarray(ce[b0:b0 + BL].T)
        in_maps.append(m)
    res = run_bass_kernel_spmd(nc, in_maps, core_ids=list(range(NC_)))
    _LAST_EXEC_NS = res.exec_time_ns
    out = np.concatenate([res.results[c]["out"] for c in range(NC_)], axis=0)
    gl = np.concatenate([res.results[c]["gate_out"] for c in range(NC_)], axis=0)
    return out, gl


# revision 21
# speedup vs baseline: 1.2874x; 1.2874x over previous
"""Trainium2 Bass kernel for nn_CLIPVisionTowerContextMoe.

Data-parallel over batch B=16 across 8 NeuronCores (2 batches/core, no
collectives). All matmuls in float32r (TF32-like, 1 cyc/row at N>=256).
Feature-major layouts throughout; host pre-transposes weights/activations.
Tokens padded 577 -> 640 per batch (clean 512/256 moving tiles).

Per-core pipeline:
  gate path:  txt resblock -> q/k-bias/v-bias -> per-320-token-tile img
              resblock -> k+scores / v+online-attention (unnormalized exp,
              no max pass) -> task -> gate logits/probs
  expert path: 32 phases of (expert e, 4-f-chunk block): MM1 + fused
              bias+QuickGELU (Sigmoid ACT + scalar_tensor_tensor) ->
              token-major MM2 (swapped operands) -> fused scaled flush
              out_acc += g[b,e] * psum.  Expert bias b2 folded in via
              out_acc init from broadcast gp@exp_b2 rows.
"""
import os
import sys

# The kernel executes through the axon PJRT backend; a JAX_PLATFORMS=cpu pin
# (common for reference computation) would hide the 8 NeuronCores. Clear it
# before jax's backend initializes (no-op if jax is already initialized).
if os.environ.get("JAX_PLATFORMS") == "cpu" and "jax" not in sys.modules:
    os.environ.pop("JAX_PLATFORMS")

import numpy as np
from contextlib import ExitStack

import concourse.bacc as bacc
import concourse.mybir as mybir
import concourse.tile as tile
from concourse.bass_utils import run_bass_kernel_spmd

F32 = mybir.dt.float32
F32R = mybir.dt.float32r
AF = mybir.ActivationFunctionType
OP = mybir.AluOpType
AX = mybir.AxisListType

B, N, D, C, E, F, H = 16, 577, 1024, 768, 4, 4096, 4
NPAD, BL = 640, 2
T = BL * NPAD            # 1280
HD = C // H              # 192
F2, C2 = 2 * D, 2 * C    # 2048, 1536
NC_ = 8                  # cores
DC_, CC_, FC_, F2C, C2C = D // 128, C // 128, F // 128, F2 // 128, C2 // 128
NT = 320                 # gate-phase token tile (4 tiles, no batch crossing)
SCALE_ATT = 1.0 / np.sqrt(HD)
QG = 1.702
TBLK = [(0, 512), (512, 1024), (1024, 1280)]   # expert MM1 moving tiles
NTC = T // 128           # 10 token chunks for MM2/out
NF = 4                   # f-chunks per expert phase


def _build():
    nc = bacc.Bacc("TRN2", num_devices=NC_)
    dt = nc.dram_tensor

    # ---- inputs (per core). f32r for matmul-consumed tensors. ----
    xt = dt("xt", [D, T], F32R, kind="ExternalInput")
    ctxt = dt("ctxt", [C, BL], F32R, kind="ExternalInput")
    w1i = dt("w1i", [D, F2], F32R, kind="ExternalInput")      # img_fc1_w.T
    w2i = dt("w2i", [F2, D], F32R, kind="ExternalInput")      # img_fc2_w.T
    w1t = dt("w1t", [C, C2], F32R, kind="ExternalInput")
    w2t = dt("w2t", [C2, C], F32R, kind="ExternalInput")
    qw = dt("qw", [C, C], F32R, kind="ExternalInput")
    kwi = dt("kwi", [D, C], F32R, kind="ExternalInput")
    kwt = dt("kwt", [C, C], F32R, kind="ExternalInput")
    vwi = dt("vwi", [D, C], F32R, kind="ExternalInput")
    vwt = dt("vwt", [C, C], F32R, kind="ExternalInput")
    aow = dt("aow", [C, C], F32R, kind="ExternalInput")
    gw = dt("gw", [C, E], F32R, kind="ExternalInput")
    ew1 = dt("ew1", [E, D, F], F32R, kind="ExternalInput")    # exp_w1 transposed
    ew2 = dt("ew2", [E, F, D], F32R, kind="ExternalInput")    # exp_w2 transposed
    b2stack = dt("b2stack", [E, D], F32R, kind="ExternalInput")
    b1i_c = dt("b1i_c", [128, F2C], F32, kind="ExternalInput")
    b1i_s = dt("b1i_s", [128, F2C], F32, kind="ExternalInput")
    b2i_c = dt("b2i_c", [128, DC_], F32, kind="ExternalInput")
    gi_c = dt("gi_c", [128, DC_], F32, kind="ExternalInput")
    bi_c = dt("bi_c", [128, DC_], F32, kind="ExternalInput")
    b1t_c = dt("b1t_c", [128, C2C], F32, kind="ExternalInput")
    b1t_s = dt("b1t_s", [128, C2C], F32, kind="ExternalInput")
    b2t_c = dt("b2t_c", [128, CC_], F32, kind="ExternalInput")
    gt_c = dt("gt_c", [128, CC_], F32, kind="ExternalInput")
    bt_c = dt("bt_c", [128, CC_], F32, kind="ExternalInput")
    qb_c = dt("qb_c", [128, CC_], F32, kind="ExternalInput")
    kb_c = dt("kb_c", [128, CC_], F32, kind="ExternalInput")
    vb_c = dt("vb_c", [128, CC_], F32, kind="ExternalInput")
    aob_c = dt("aob_c", [128, CC_], F32, kind="ExternalInput")
    gb_r = dt("gb_r", [BL, E], F32, kind="ExternalInput")
    eb1_c = dt("eb1_c", [128, E, FC_], F32, kind="ExternalInput")
    eb1_s = dt("eb1_s", [128, E, FC_], F32, kind="ExternalInput")
    ones_col_d = dt("ones_col", [128, 1], F32R, kind="ExternalInput")
    ones_row_d = dt("ones_row", [1, 128], F32R, kind="ExternalInput")
    ind4_d = dt("ind4", [128, CC_, H], F32, kind="ExternalInput")
    ind4t_d = dt("ind4t", [H, C], F32R, kind="ExternalInput")

    out = dt("out", [BL, N, D], F32, kind="ExternalOutput")
    gate_out = dt("gate_out", [BL, E], F32, kind="ExternalOutput")

    scr_gp = dt("scr_gp", [BL, E], F32R, kind="Internal")
    scr_b2m = dt("scr_b2m", [BL, D], F32R, kind="Internal")

    with nc.allow_low_precision("float32r tiles feed f32r matmuls by design"), \
            tile.TileContext(nc) as tc, \
            ExitStack() as ctx:
        persist = ctx.enter_context(tc.tile_pool(name="persist", bufs=1))
        big = ctx.enter_context(tc.tile_pool(name="big", bufs=1))
        w2rp = ctx.enter_context(tc.tile_pool(name="w2rp", bufs=1))
        stream = ctx.enter_context(tc.tile_pool(name="stream", bufs=4))
        gbuf = ctx.enter_context(tc.tile_pool(name="gbuf", bufs=1))
        trans = ctx.enter_context(tc.tile_pool(name="trans", bufs=2))
        ps = ctx.enter_context(tc.tile_pool(name="ps", bufs=8, space="PSUM"))

        def pst(p_, n_):
            return ps.tile([p_, n_], F32, tag="ps", name="pst")

        # ---- resident loads ----
        xt_sb = persist.tile([128, DC_, T], F32R)
        nc.sync.dma_start(xt_sb, xt.ap().rearrange("(dc p) t -> p dc t", p=128))
        out_acc = persist.tile([128, NTC, D], F32)
        eb1c_sb = persist.tile([128, E, FC_], F32)
        nc.sync.dma_start(eb1c_sb, eb1_c.ap())
        eb1s_sb = persist.tile([128, E, FC_], F32)
        nc.sync.dma_start(eb1s_sb, eb1_s.ap())
        ones_col = persist.tile([128, 1], F32R)
        nc.sync.dma_start(ones_col, ones_col_d.ap())
        ones_row = persist.tile([1, 128], F32R)
        nc.sync.dma_start(ones_row, ones_row_d.ap())
        ind4 = persist.tile([128, CC_, H], F32)
        nc.sync.dma_start(ind4, ind4_d.ap())
        ind4t = persist.tile([H, C], F32R)
        nc.sync.dma_start(ind4t, ind4t_d.ap())
        gw_sb = persist.tile([128, CC_, E], F32R)
        nc.sync.dma_start(gw_sb, gw.ap().rearrange("(cc p) e -> p cc e", p=128))
        gb_sb = persist.tile([BL, E], F32)
        nc.sync.dma_start(gb_sb, gb_r.ap())
        b2stack_sb = persist.tile([E, D], F32R)
        nc.sync.dma_start(b2stack_sb, b2stack.ap())

        def load_cm(name, dram, nch):
            t_ = persist.tile([128, nch], F32, tag=name, name=name)
            nc.sync.dma_start(t_, dram.ap())
            return t_

        b1ic = load_cm("b1ic", b1i_c, F2C); b1is = load_cm("b1is", b1i_s, F2C)
        b2ic = load_cm("b2ic", b2i_c, DC_)
        gic = load_cm("gic", gi_c, DC_); bic = load_cm("bic", bi_c, DC_)
        b1tc = load_cm("b1tc", b1t_c, C2C); b1ts = load_cm("b1ts", b1t_s, C2C)
        b2tc = load_cm("b2tc", b2t_c, CC_)
        gtc = load_cm("gtc", gt_c, CC_); btc = load_cm("btc", bt_c, CC_)
        qbc = load_cm("qbc", qb_c, CC_); kbc = load_cm("kbc", kb_c, CC_)
        vbc = load_cm("vbc", vb_c, CC_); aobc = load_cm("aobc", aob_c, CC_)

        kbias = persist.tile([128, CC_, BL], F32)
        vbias = persist.tile([128, CC_, BL], F32)
        qh = persist.tile([128, CC_, BL, H], F32R)
        sumexp = persist.tile([H, BL], F32)
        av = persist.tile([128, CC_, BL], F32)
        gbc = persist.tile([128, BL * E], F32)
        nc.vector.memset(sumexp, 0.0)
        nc.vector.memset(av, 0.0)
        eps_t = persist.tile([128, 1], F32)
        nc.vector.memset(eps_t, 1e-5)

        def stream_w(dram_ap, kch, col0, ncols=128, k0=0):
            """lhsT block [128, kch, ncols] from a [K, M] dram weight (shared tag)."""
            t_ = stream.tile([128, kch, ncols], F32R, tag="ws", name="ws")
            nc.sync.dma_start(
                t_, dram_ap.rearrange("(kc p) m -> p kc m", p=128)
                [:, k0:k0 + kch, col0:col0 + ncols])
            return t_

        # ============ txt resblock + q/kbias/vbias (feature-major, N=2) ============
        ctx_sb = persist.tile([128, CC_, BL], F32R)
        nc.sync.dma_start(ctx_sb, ctxt.ap().rearrange("(cc p) b -> p cc b", p=128))

        ht = trans.tile([128, C2C, BL], F32R, tag="ht")
        for c2 in range(C2C):
            w_ = stream_w(w1t.ap(), CC_, c2 * 128)
            z = pst(128, BL)
            for cc in range(CC_):
                nc.tensor.matmul(z, w_[:, cc, :], ctx_sb[:, cc, :],
                                 start=(cc == 0), stop=(cc == CC_ - 1))
            s_ = trans.tile([128, BL], F32, tag="sig")
            nc.scalar.activation(s_, z, AF.Sigmoid, bias=b1ts[:, c2:c2 + 1], scale=QG)
            nc.vector.scalar_tensor_tensor(ht[:, c2, :], z, b1tc[:, c2:c2 + 1], s_,
                                           op0=OP.add, op1=OP.mult)
        yt = trans.tile([128, CC_, BL], F32R, tag="yt")
        ps_s = pst(1, BL)
        ps_q = pst(1, BL)
        for cc in range(CC_):
            z = pst(128, BL)
            for ch in range(2):
                w_ = stream_w(w2t.ap(), C2C // 2, cc * 128, k0=ch * (C2C // 2))
                for c2 in range(C2C // 2):
                    nc.tensor.matmul(z, w_[:, c2, :], ht[:, ch * (C2C // 2) + c2, :],
                                     start=(ch == 0 and c2 == 0),
                                     stop=(ch == 1 and c2 == C2C // 2 - 1))
            nc.vector.scalar_tensor_tensor(yt[:, cc, :], z, b2tc[:, cc:cc + 1],
                                           ctx_sb[:, cc, :], op0=OP.add, op1=OP.add)
            sqt = trans.tile([128, BL], F32R, tag="sqt")
            nc.scalar.activation(sqt, yt[:, cc, :], AF.Square)
            nc.tensor.matmul(ps_s, ones_col, yt[:, cc, :],
                             start=(cc == 0), stop=(cc == CC_ - 1))
            nc.tensor.matmul(ps_q, ones_col, sqt,
                             start=(cc == 0), stop=(cc == CC_ - 1))

        def ln_rows(ps_s_, ps_q_, nfeat, width):
            """(rstd_bc, nb_bc) psum [128, width] from sum/sumsq row psums."""
            m_ = trans.tile([1, width], F32, tag="lnr1", name="m_")
            nc.scalar.mul(m_, ps_s_, 1.0 / nfeat)
            m2 = trans.tile([1, width], F32, tag="lnr2", name="m2")
            nc.vector.tensor_mul(m2, m_, m_)
            var = trans.tile([1, width], F32, tag="lnr3", name="var")
            nc.vector.scalar_tensor_tensor(var, ps_q_, 1.0 / nfeat, m2,
                                           op0=OP.mult, op1=OP.subtract)
            sd = trans.tile([1, width], F32, tag="lnr4", name="sd")
            nc.scalar.activation(sd, var, AF.Sqrt, bias=eps_t[0:1, :])
            rstd = trans.tile([1, width], F32R, tag="lnr5", name="rstd")
            nc.vector.reciprocal(rstd, sd)
            nb = trans.tile([1, width], F32R, tag="lnr6", name="nb")
            nc.vector.scalar_tensor_tensor(nb, m_, -1.0, rstd, op0=OP.mult, op1=OP.mult)
            bc_r = pst(128, width)
            nc.tensor.matmul(bc_r, ones_row, rstd, start=True, stop=True)
            bc_n = pst(128, width)
            nc.tensor.matmul(bc_n, ones_row, nb, start=True, stop=True)
            return bc_r, bc_n

        bc_r, bc_n = ln_rows(ps_s, ps_q, C, BL)
        txt_sb = persist.tile([128, CC_, BL], F32R)
        for cc in range(CC_):
            t1 = trans.tile([128, BL], F32, tag="t1")
            nc.vector.tensor_mul(t1, yt[:, cc, :], bc_r)
            nc.vector.tensor_add(t1, t1, bc_n)
            nc.vector.tensor_scalar(txt_sb[:, cc, :], t1, gtc[:, cc:cc + 1],
                                    btc[:, cc:cc + 1], op0=OP.mult, op1=OP.add)

        def proj_c(w_dram, bias_cm, out_t):
            for co in range(CC_):
                w_ = stream_w(w_dram.ap(), CC_, co * 128)
                z = pst(128, BL)
                for ci in range(CC_):
                    nc.tensor.matmul(z, w_[:, ci, :], txt_sb[:, ci, :],
                                     start=(ci == 0), stop=(ci == CC_ - 1))
                nc.scalar.activation(out_t[:, co, :], z, AF.Identity,
                                     bias=bias_cm[:, co:co + 1])

        qt = persist.tile([128, CC_, BL], F32)
        proj_c(qw, qbc, qt)
        proj_c(kwt, kbc, kbias)
        proj_c(vwt, vbc, vbias)

        for cc in range(CC_):
            for b in range(BL):
                nc.vector.tensor_scalar_mul(qh[:, cc, b, :], ind4[:, cc, :],
                                            qt[:, cc, b:b + 1])

        # ============ img resblock + k/scores + v/attention per 320-token tile ========
        for it in range(4):
            b = it // 2
            t0 = it * NT
            xsl = xt_sb[:, :, t0:t0 + NT]
            real = NT if it % 2 == 0 else N - NT   # 320 or 257

            h_img = big.tile([128, F2C, NT], F32R, tag="big", name="h_img")
            for f2 in range(F2C):
                w_ = stream_w(w1i.ap(), DC_, f2 * 128)
                z = pst(128, NT)
                for dc in range(DC_):
                    nc.tensor.matmul(z, w_[:, dc, :], xsl[:, dc, :],
                                     start=(dc == 0), stop=(dc == DC_ - 1))
                s_ = trans.tile([128, NT], F32, tag="sigi")
                nc.scalar.activation(s_, z, AF.Sigmoid, bias=b1is[:, f2:f2 + 1], scale=QG)
                nc.vector.scalar_tensor_tensor(h_img[:, f2, :], z, b1ic[:, f2:f2 + 1],
                                               s_, op0=OP.add, op1=OP.mult)

            y = gbuf.tile([128, DC_, NT], F32R, tag="y_img", name="y")
            ps_s2 = pst(1, NT)
            ps_q2 = pst(1, NT)
            for dc in range(DC_):
                z = pst(128, NT)
                for ch in range(2):
                    wa = stream_w(w2i.ap(), F2C // 2, dc * 128, k0=ch * (F2C // 2))
                    for f2 in range(F2C // 2):
                        nc.tensor.matmul(z, wa[:, f2, :],
                                         h_img[:, ch * (F2C // 2) + f2, :],
                                         start=(ch == 0 and f2 == 0),
                                         stop=(ch == 1 and f2 == F2C // 2 - 1))
                nc.vector.scalar_tensor_tensor(y[:, dc, :], z, b2ic[:, dc:dc + 1],
                                               xsl[:, dc, :], op0=OP.add, op1=OP.add)
                sq = trans.tile([128, NT], F32R, tag="sq")
                nc.scalar.activation(sq, y[:, dc, :], AF.Square)
                nc.tensor.matmul(ps_s2, ones_col, y[:, dc, :],
                                 start=(dc == 0), stop=(dc == DC_ - 1))
                nc.tensor.matmul(ps_q2, ones_col, sq,
                                 start=(dc == 0), stop=(dc == DC_ - 1))
            bc_r2, bc_n2 = ln_rows(ps_s2, ps_q2, D, NT)
            imgn = gbuf.tile([128, DC_, NT], F32R, tag="imgn", name="imgn")
            for dc in range(DC_):
                t2 = trans.tile([128, NT], F32, tag="t2")
                nc.vector.tensor_mul(t2, y[:, dc, :], bc_r2)
                nc.vector.tensor_add(t2, t2, bc_n2)
                nc.vector.tensor_scalar(imgn[:, dc, :], t2, gic[:, dc:dc + 1],
                                        bic[:, dc:dc + 1], op0=OP.mult, op1=OP.add)

            # pass 1: k per c-chunk + scores accumulation
            z_sc = pst(H, NT)
            for cc in range(CC_):
                w_ = stream_w(kwi.ap(), DC_, cc * 128)
                zk = pst(128, NT)
                for dc in range(DC_):
                    nc.tensor.matmul(zk, w_[:, dc, :], imgn[:, dc, :],
                                     start=(dc == 0), stop=(dc == DC_ - 1))
                k_cc = trans.tile([128, NT], F32R, tag="k_cc")
                nc.scalar.activation(k_cc, zk, AF.Identity, bias=kbias[:, cc, b:b + 1])
                nc.tensor.matmul(z_sc, qh[:, cc, b, :], k_cc,
                                 start=(cc == 0), stop=(cc == CC_ - 1))
            ex = trans.tile([H, NT], F32R, tag="ex")
            nc.scalar.activation(ex, z_sc, AF.Exp, scale=SCALE_ATT)
            red4 = trans.tile([H, 1], F32, tag="red4")
            nc.vector.tensor_reduce(red4, ex[:, :real], axis=AX.X, op=OP.add)
            nc.vector.tensor_add(sumexp[:, b:b + 1], sumexp[:, b:b + 1], red4)

            # pass 2: v per c-chunk + online attention-value accumulation
            for cc in range(CC_):
                w_ = stream_w(vwi.ap(), DC_, cc * 128)
                zv = pst(128, NT)
                for dc in range(DC_):
                    nc.tensor.matmul(zv, w_[:, dc, :], imgn[:, dc, :],
                                     start=(dc == 0), stop=(dc == DC_ - 1))
                v_cc = trans.tile([128, NT], F32R, tag="v_cc")
                nc.scalar.activation(v_cc, zv, AF.Identity, bias=vbias[:, cc, b:b + 1])
                zb = pst(128, NT)
                nc.tensor.matmul(zb, ind4t[:, cc * 128:(cc + 1) * 128],
                                 ex, start=True, stop=True)
                prod = trans.tile([128, NT], F32, tag="prod")
                nc.vector.tensor_mul(prod[:, :real], v_cc[:, :real], zb[:, :real])
                red = trans.tile([128, 1], F32, tag="red")
                nc.vector.tensor_reduce(red, prod[:, :real], axis=AX.X, op=OP.add)
                nc.vector.tensor_add(av[:, cc, b:b + 1], av[:, cc, b:b + 1], red)

        # ============ attention tail: avn -> task -> gate ============
        recip = trans.tile([H, BL], F32R, tag="recip")
        nc.vector.reciprocal(recip, sumexp)
        avn = persist.tile([128, CC_, BL], F32R)
        for cc in range(CC_):
            zr = pst(128, BL)
            nc.tensor.matmul(zr, ind4t[:, cc * 128:(cc + 1) * 128], recip,
                             start=True, stop=True)
            nc.vector.tensor_mul(avn[:, cc, :], av[:, cc, :], zr)
        taskT = persist.tile([128, CC_, BL], F32R)
        for co in range(CC_):
            w_ = stream_w(aow.ap(), CC_, co * 128)
            z = pst(128, BL)
            for ci in range(CC_):
                nc.tensor.matmul(z, w_[:, ci, :], avn[:, ci, :],
                                 start=(ci == 0), stop=(ci == CC_ - 1))
            nc.scalar.activation(taskT[:, co, :], z, AF.Identity,
                                 bias=aobc[:, co:co + 1])
        z_g = pst(BL, E)
        for cc in range(CC_):
            nc.tensor.matmul(z_g, taskT[:, cc, :], gw_sb[:, cc, :],
                             start=(cc == 0), stop=(cc == CC_ - 1))
        gl = trans.tile([BL, E], F32, tag="gl")
        nc.vector.tensor_add(gl, z_g, gb_sb)
        nc.sync.dma_start(gate_out.ap(), gl)
        uexp = trans.tile([BL, E], F32, tag="uexp")
        nc.scalar.activation(uexp, gl, AF.Exp)
        srow = trans.tile([BL, 1], F32, tag="srow")
        nc.vector.tensor_reduce(srow, uexp, axis=AX.X, op=OP.add)
        rrow = trans.tile([BL, 1], F32, tag="rrow")
        nc.vector.reciprocal(rrow, srow)
        gp = trans.tile([BL, E], F32R, tag="gp")
        nc.vector.tensor_scalar_mul(gp, uexp, rrow)
        nc.sync.dma_start(scr_gp.ap(), gp)
        gp_row = trans.tile([1, BL, E], F32R, tag="gp_row")
        nc.sync.dma_start(gp_row, scr_gp.ap().rearrange("b e -> () b e"))
        gpT = trans.tile([E, BL], F32R, tag="gpT")
        nc.sync.dma_start(gpT, scr_gp.ap().rearrange("b e -> e b"))
        z_gbc = pst(128, BL * E)
        nc.tensor.matmul(z_gbc, ones_row, gp_row, start=True, stop=True)
        nc.scalar.copy(gbc, z_gbc)
        b2m = gbuf.tile([BL, D], F32R, tag="y_img", name="b2m")
        for dh in range(2):
            z_b2 = pst(BL, 512)
            nc.tensor.matmul(z_b2, gpT, b2stack_sb[:, dh * 512:(dh + 1) * 512],
                             start=True, stop=True)
            nc.scalar.copy(b2m[:, dh * 512:(dh + 1) * 512], z_b2)
        nc.sync.dma_start(scr_b2m.ap(), b2m)
        b2row = gbuf.tile([1, BL, D], F32R, tag="imgn", name="b2row")
        nc.sync.dma_start(b2row, scr_b2m.ap().rearrange("b d -> () b d"))
        for q4 in range(4):
            b, dh = q4 // 2, q4 % 2
            z_bb = pst(128, 512)
            nc.tensor.matmul(z_bb, ones_row, b2row[:, b, dh * 512:dh * 512 + 512],
                             start=True, stop=True)
            for tc_ in range(b * (NTC // 2), (b + 1) * (NTC // 2)):
                nc.vector.tensor_copy(out_acc[:, tc_, dh * 512:(dh + 1) * 512], z_bb)

        # ============ experts: 32 phases of (e, 4-f-chunk block) ============
        for e in range(E):
            for fh in range(FC_ // NF):
                h_exp = big.tile([128, NF, T], F32R, tag="big", name="h_exp")
                w2r = w2rp.tile([128, NF, D], F32R, name="w2r")
                nc.sync.dma_start(
                    w2r, ew2.ap()[e].rearrange("(fb p) d -> p fb d", p=128)
                    [:, fh * NF:(fh + 1) * NF, :])
                for fi in range(NF):
                    fc = fh * NF + fi
                    w_ = stream_w(ew1.ap()[e], DC_, fc * 128)
                    zs = [pst(128, tb - ta) for (ta, tb) in TBLK]
                    for dc in range(DC_):
                        for k, (ta, tb) in enumerate(TBLK):
                            nc.tensor.matmul(zs[k], w_[:, dc, :], xt_sb[:, dc, ta:tb],
                                             start=(dc == 0), stop=(dc == DC_ - 1))
                    for k, (ta, tb) in enumerate(TBLK):
                        s_ = trans.tile([128, 512], F32, tag="sige")
                        nc.scalar.activation(s_[:, :tb - ta], zs[k], AF.Sigmoid,
                                             bias=eb1s_sb[:, e, fc:fc + 1], scale=QG)
                        nc.vector.scalar_tensor_tensor(
                            h_exp[:, fi, ta:tb], zs[k], eb1c_sb[:, e, fc:fc + 1],
                            s_[:, :tb - ta], op0=OP.add, op1=OP.mult)
                for tc_ in range(NTC):
                    b = tc_ // (NTC // 2)
                    gsc = gbc[:, b * E + e:b * E + e + 1]
                    for dh in range(2):
                        z = pst(128, 512)
                        for fi in range(NF):
                            nc.tensor.matmul(z, h_exp[:, fi, tc_ * 128:(tc_ + 1) * 128],
                                             w2r[:, fi, dh * 512:(dh + 1) * 512],
                                             start=(fi == 0), stop=(fi == NF - 1))
                        sl = out_acc[:, tc_, dh * 512:(dh + 1) * 512]
                        nc.vector.scalar_tensor_tensor(sl, z, gsc, sl,
                                                       op0=OP.mult, op1=OP.add)

        # ---- write outputs ----
        for tc_ in range(NTC):
            b = tc_ // (NTC // 2)
            n0 = (tc_ % (NTC // 2)) * 128
            rows = min(128, N - n0)
            if rows <= 0:
                continue
            nc.sync.dma_start(out.ap()[b, n0:n0 + rows, :], out_acc[:rows, tc_, :])

    nc.finalize()
    return nc


_NC_CACHE = {}


def _get_nc():
    if "nc" not in _NC_CACHE:
        _NC_CACHE["nc"] = _build()
    return _NC_CACHE["nc"]


def _prep_shared(w):
    """Host-side weight marshalling (shared across cores)."""
    f32 = np.float32
    c = lambda a: np.ascontiguousarray(a, dtype=f32)
    cm = lambda v, nch: c(np.asarray(v, f32).reshape(nch, 128).T)
    head_of = np.arange(C) // HD
    ind4 = np.zeros((128, CC_, H), f32)
    for cc in range(CC_):
        p = np.arange(128) + cc * 128
        ind4[np.arange(128), cc, head_of[p]] = 1.0
    ind4t = np.zeros((H, C), f32)
    ind4t[head_of, np.arange(C)] = 1.0
    return {
        "w1i": c(np.asarray(w["img_fc1_w"]).T), "w2i": c(np.asarray(w["img_fc2_w"]).T),
        "w1t": c(np.asarray(w["txt_fc1_w"]).T), "w2t": c(np.asarray(w["txt_fc2_w"]).T),
        "qw": c(np.asarray(w["q_w"]).T),
        "kwi": c(np.asarray(w["k_w"])[:, :D].T), "kwt": c(np.asarray(w["k_w"])[:, D:].T),
        "vwi": c(np.asarray(w["v_w"])[:, :D].T), "vwt": c(np.asarray(w["v_w"])[:, D:].T),
        "aow": c(np.asarray(w["attn_out_w"]).T), "gw": c(np.asarray(w["gate_w"]).T),
        "ew1": c(np.asarray(w["exp_w1"]).transpose(0, 2, 1)),
        "ew2": c(np.asarray(w["exp_w2"]).transpose(0, 2, 1)),
        "b2stack": c(w["exp_b2"]),
        "b1i_c": cm(w["img_fc1_b"], F2C), "b1i_s": cm(np.asarray(w["img_fc1_b"]) * QG, F2C),
        "b2i_c": cm(w["img_fc2_b"], DC_),
        "gi_c": cm(w["img_ln_g"], DC_), "bi_c": cm(w["img_ln_b"], DC_),
        "b1t_c": cm(w["txt_fc1_b"], C2C), "b1t_s": cm(np.asarray(w["txt_fc1_b"]) * QG, C2C),
        "b2t_c": cm(w["txt_fc2_b"], CC_),
        "gt_c": cm(w["txt_ln_g"], CC_), "bt_c": cm(w["txt_ln_b"], CC_),
        "qb_c": cm(w["q_b"], CC_), "kb_c": cm(w["k_b"], CC_), "vb_c": cm(w["v_b"], CC_),
        "aob_c": cm(w["attn_out_b"], CC_),
        "gb_r": c(np.broadcast_to(np.asarray(w["gate_b"], f32)[None, :], (BL, E))),
        "eb1_c": c(np.asarray(w["exp_b1"], f32).reshape(E, FC_, 128).transpose(2, 0, 1)),
        "eb1_s": c((np.asarray(w["exp_b1"], f32) * QG).reshape(E, FC_, 128).transpose(2, 0, 1)),
        "ones_col": np.ones((128, 1), f32), "ones_row": np.ones((1, 128), f32),
        "ind4": ind4, "ind4t": ind4t,
    }


_LAST_EXEC_NS = None


def kernel(**inputs):
    global _LAST_EXEC_NS
    nc = _get_nc()
    shared = _prep_shared(inputs)
    hs = np.asarray(inputs["hidden_states"], np.float32)
    ce = np.asarray(inputs["context_embeddings"], np.float32)
    in_maps = []
    for core in range(NC_):
        b0 = core * BL
        xT = np.zeros((D, T), np.float32)
        for b in range(BL):
            xT[:, b * NPAD:b * NPAD + N] = hs[b0 + b].T
        m = dict(shared)
        m["xt"] = xT
        m["ctxt"] = np.ascontiguousarray(ce[b0:b0 + BL].T)
        in_maps.append(m)
    res = run_bass_kernel_spmd(nc, in_maps, core_ids=list(range(NC_)))
    _LAST_EXEC_NS = res.exec_time_ns
    out = np.concatenate([res.results[c]["out"] for c in range(NC_)], axis=0)
    gl = np.concatenate([res.results[c]["gate_out"] for c in range(NC_)], axis=0)
    return out, gl
